# revision 1
# baseline (speedup 1.0000x reference)
"""Single-head cross-attention (layernorm + QKV proj + softmax(QK^T)V) on 8 NeuronCores.

Sharding: data-parallel over batch B=8, one batch element per core.

Per-core device program (all matmuls bf16 with fp32 PSUM accumulation):
  1. Layernorm (no affine; g/b folded into weights on host) of target/source_k/source_v
     in natural [token, d] layout via bn_stats, output bf16.
  2. DMA-xbar transpose of normalized activations to [d, token] layout.
  3. Projections with pre-transposed weights produce qT/kT in [e, token] layout and
     v in natural [token, e] layout (so no further transposes are needed).
  4. scores^T[j, i] = kT^T qT, exp via ScalarE (scale 1/sqrt(d) fused, no max
     subtraction: |scores*scale| < 3), giving unnormalized attn^T in bf16.
  5. out_u[i, e] = attn^T.T @ v and Z[i] = attn^T.T @ ones accumulate in PSUM;
     out = out_u / Z.
"""

import os
from contextlib import ExitStack

import numpy as np
import ml_dtypes

import concourse.bass as bass
import concourse.bacc as bacc
import concourse.mybir as mybir
import concourse.tile as tile
from concourse.bass import ts, ds
from concourse.bass_utils import run_bass_kernel_spmd

BF16 = mybir.dt.bfloat16
F32 = mybir.dt.float32

B, T, D = 8, 2048, 1024
EPS = 1e-5
SCALE = float(D) ** -0.5
P = 128
N_IT = T // P          # 16 token tiles of 128
N_DB = D // P          # 8 d-blocks of 128
N_EB = D // P          # 8 e-blocks of 128
N_IC = T // 512        # 4 token chunks of 512
N_EC = D // 512        # 2 e chunks of 512


def _ln_transpose(nc, pool_stage, streams, eps_t):
    """Layernorm + DMA-xbar transpose for one or more tensors, interleaved.

    streams: list of (name, x_dram, xt_tile). Interleaving independent chains
    keeps the in-order DVE queue busy while each chain crosses to ACT for the
    fused 1/sqrt(var+eps) and back. Software-pipelined by one tile.
    """
    tiles = {}

    def stats_stage(s, it):
        name, tb, x_dram, xt_tile = streams[s]
        x_raw = pool_stage.tile([P, D], F32, tag=f"x_raw_{tb}", bufs=2,
                                name=f"x_raw_{name}_{it}")
        nc.gpsimd.dma_start(out=x_raw, in_=x_dram[ts(it, P), :])
        stats = pool_stage.tile([P, 2, 6], F32, tag=f"stats_{tb}", bufs=3,
                                name=f"stats_{name}_{it}")
        for sb in range(2):
            nc.vector.bn_stats(out=stats[:, sb, :], in_=x_raw[:, ts(sb, 512)])
        mv = pool_stage.tile([P, 2], F32, tag=f"mv_{tb}", bufs=3, name=f"mv_{name}_{it}")
        nc.vector.bn_aggr(out=mv, in_=stats)
        rstd = pool_stage.tile([P, 1], F32, tag=f"rstd_{tb}", bufs=3,
                               name=f"rstd_{name}_{it}")
        nc.scalar.activation(
            out=rstd, in_=mv[:, 1:2],
            func=mybir.ActivationFunctionType.Abs_reciprocal_sqrt,
            bias=eps_t,
        )
        tiles[(s, it)] = (x_raw, mv, rstd)

    def apply_stage(s, it):
        name, tb, x_dram, xt_tile = streams[s]
        x_raw, mv, rstd = tiles.pop((s, it))
        ln_out = pool_stage.tile([P, D], BF16, tag=f"ln_out_{tb}", bufs=2,
                                 name=f"ln_out_{name}_{it}")
        nc.vector.tensor_scalar(
            out=ln_out, in0=x_raw, scalar1=mv[:, 0:1], scalar2=rstd,
            op0=mybir.AluOpType.subtract, op1=mybir.AluOpType.mult,
        )
        nc.sync.dma_start(out=xt_tile[:, it], in_=ln_out, transpose=True)

    ns = len(streams)
    for it in range(N_IT):
        for s in range(ns):
            stats_stage(s, it)
        if it > 0:
            for s in range(ns):
                apply_stage(s, it - 1)
    for s in range(ns):
        apply_stage(s, N_IT - 1)


def _xt_rhs(xt_tile, db, ic):
    """Moving operand [d-block partition, 512 tokens] for token chunk ic."""
    return xt_tile[:, ds(ic * 4, 4), db, :]


def build_module() -> bass.Bass:
    nc = bacc.Bacc("TRN2", target_bir_lowering=False)

    x_t = nc.dram_tensor("x_t", [T, D], F32, kind="ExternalInput")
    x_k = nc.dram_tensor("x_k", [T, D], F32, kind="ExternalInput")
    x_v = nc.dram_tensor("x_v", [T, D], F32, kind="ExternalInput")
    wq = nc.dram_tensor("wq", [D, D], BF16, kind="ExternalInput")  # pre-transposed [d, e]
    wk = nc.dram_tensor("wk", [D, D], BF16, kind="ExternalInput")
    wv = nc.dram_tensor("wv", [D, D], BF16, kind="ExternalInput")
    bq = nc.dram_tensor("bq", [D], F32, kind="ExternalInput")
    bk = nc.dram_tensor("bk", [D], F32, kind="ExternalInput")
    bv = nc.dram_tensor("bv", [D], F32, kind="ExternalInput")
    out = nc.dram_tensor("out", [T, D], F32, kind="ExternalOutput")

    with tile.TileContext(nc) as tc, ExitStack() as ctx:
        const = ctx.enter_context(tc.tile_pool(name="const", bufs=1))
        qkv = ctx.enter_context(tc.tile_pool(name="qkv", bufs=1))
        mm_ps = ctx.enter_context(tc.tile_pool(name="mm_ps", bufs=4, space="PSUM"))

        # ---- constants ----
        eps_t = const.tile([P, 1], F32)
        nc.vector.memset(eps_t, EPS)
        ones_t = const.tile([P, 1], BF16)
        nc.vector.memset(ones_t, 1.0)
        bq_sb = const.tile([P, N_EB], F32)
        nc.gpsimd.dma_start(out=bq_sb, in_=bq[:].rearrange("(a p) -> p a", p=P))
        bk_sb = const.tile([P, N_EB], F32)
        nc.gpsimd.dma_start(out=bk_sb, in_=bk[:].rearrange("(a p) -> p a", p=P))
        bv_ap = bv[:]
        bv_bc = const.tile([P, D], F32)
        nc.gpsimd.dma_start(
            out=bv_bc,
            in_=bass.AP(tensor=bv_ap.tensor, offset=bv_ap.offset,
                        ap=[[0, P]] + list(bv_ap.ap)),
        )

        # ---- persistent projection outputs ----
        qT = qkv.tile([P, N_EB, T], BF16)        # qT[p, eb, i] = q[i, eb*128+p]
        kT = qkv.tile([P, N_EB, T], BF16)
        v_sb = qkv.tile([P, N_IT, D], BF16)      # v[p, jt, e] = v[jt*128+p, e]

        with tc.tile_pool(name="proj_phase", bufs=1) as pp:
            def projectT(w_dram, bias_sb, xt_tile, dstT, pname):
                """dstT[p, eb, i] = sum_d ln[i, d] * w[d, eb*128+p] + bias."""
                for ic in range(N_IC):
                    for eb in range(N_EB):
                        w_sl = pp.tile([P, N_DB, P], BF16, tag="w_sl", bufs=3,
                                       name=f"w_{pname}_{ic}_{eb}")
                        nc.sync.dma_start(
                            out=w_sl,
                            in_=w_dram[:, ts(eb, P)].rearrange("(a p) e -> p a e", p=P),
                        )
                        ps = mm_ps.tile([P, 512], F32, tag="mm", name=f"ps_{pname}_{eb}_{ic}")
                        for db in range(N_DB):
                            nc.tensor.matmul(
                                ps, lhsT=w_sl[:, db, :],
                                rhs=_xt_rhs(xt_tile, db, ic),
                                start=(db == 0), stop=(db == N_DB - 1),
                            )
                        nc.scalar.activation(
                            out=dstT[:, eb, ts(ic, 512)], in_=ps,
                            func=mybir.ActivationFunctionType.Identity,
                            bias=bias_sb[:, eb:eb + 1],
                        )

            # target -> qT
            with nc.named_scope("ln_t"):
                xt_t = pp.tile([P, N_IT, N_DB, P], BF16, tag="xt", bufs=2)
                _ln_transpose(nc, pp, [("t", "a", x_t, xt_t)], eps_t)
            with nc.named_scope("proj_q"):
                projectT(wq, bq_sb, xt_t, qT, "q")

            # source_k -> kT
            with nc.named_scope("ln_k"):
                xt_k = pp.tile([P, N_IT, N_DB, P], BF16, tag="xt", bufs=2)
                _ln_transpose(nc, pp, [("k", "b", x_k, xt_k)], eps_t)
            with nc.named_scope("proj_k"):
                projectT(wk, bk_sb, xt_k, kT, "k")

            # source_v -> v (natural layout): v[jt, e] = sum_d ln_v[j, d] w_v[d, e] + bv
            with nc.named_scope("ln_v"):
                xt_v = pp.tile([P, N_IT, N_DB, P], BF16, tag="xt", bufs=2)
                _ln_transpose(nc, pp, [("v", "a", x_v, xt_v)], eps_t)
            with nc.named_scope("proj_v"):
                for ec in range(N_EC):
                    w_ec = pp.tile([P, N_DB, 512], BF16, tag="wv_keep", bufs=1,
                                   name=f"wv_{ec}")
                    nc.sync.dma_start(
                        out=w_ec,
                        in_=wv[:, ts(ec, 512)].rearrange("(a p) e -> p a e", p=P),
                    )
                    for jt in range(N_IT):
                        ps = mm_ps.tile([P, 512], F32, tag="mm", name=f"ps_v_{jt}_{ec}")
                        for db in range(N_DB):
                            nc.tensor.matmul(
                                ps,
                                lhsT=xt_v[:, jt, db, :],
                                rhs=w_ec[:, db, :],
                                start=(db == 0), stop=(db == N_DB - 1),
                            )
                        nc.vector.tensor_add(
                            out=v_sb[:, jt, ts(ec, 512)], in0=ps,
                            in1=bv_bc[:, ts(ec, 512)],
                        )

        # ---- attention ----
        attv_ps = ctx.enter_context(tc.tile_pool(name="attv_ps", bufs=2, space="PSUM"))
        with tc.tile_pool(name="att", bufs=1) as att:
            for ic in range(N_IC):
                with nc.named_scope(f"scores_{ic}"):
                    aT = att.tile([P, N_IT, 512], BF16, tag="aT", bufs=2,
                                  name=f"aT_{ic}")
                    for jt in range(N_IT):
                        ps = mm_ps.tile([P, 512], F32, tag="mm", name=f"ps_s_{ic}_{jt}")
                        for eb in range(N_EB):
                            nc.tensor.matmul(
                                ps, lhsT=kT[:, eb, ts(jt, P)],
                                rhs=qT[:, eb, ts(ic, 512)],
                                start=(eb == 0), stop=(eb == N_EB - 1),
                            )
                        nc.scalar.activation(
                            out=aT[:, jt, :], in_=ps,
                            func=mybir.ActivationFunctionType.Exp, scale=SCALE,
                        )
                with nc.named_scope(f"attv_{ic}"):
                    for isub in range(4):
                        ou = attv_ps.tile([P, D], F32, tag="ou", name=f"ou_{ic}_{isub}")
                        zz = mm_ps.tile([P, 1], F32, tag="mm", name=f"z_{ic}_{isub}")
                        # same-bank runs of 16 accumulating matmuls (bank cycling
                        # between consecutive matmuls forces PE micro-stalls)
                        for ec in range(N_EC):
                            for jt in range(N_IT):
                                nc.tensor.matmul(
                                    ou[:, ts(ec, 512)], lhsT=aT[:, jt, ts(isub, P)],
                                    rhs=v_sb[:, jt, ts(ec, 512)],
                                    start=(jt == 0), stop=(jt == N_IT - 1))
                        for jt in range(N_IT):
                            nc.tensor.matmul(zz, lhsT=aT[:, jt, ts(isub, P)], rhs=ones_t,
                                             start=(jt == 0), stop=(jt == N_IT - 1))
                        rz = att.tile([P, 1], F32, tag="rz", bufs=2,
                                      name=f"rz_{ic}_{isub}")
                        nc.vector.reciprocal(out=rz, in_=zz)
                        o_sb = att.tile([P, D], F32, tag="o_sb", bufs=2,
                                        name=f"o_{ic}_{isub}")
                        nc.vector.tensor_scalar_mul(out=o_sb, in0=ou, scalar1=rz)
                        nc.sync.dma_start(out=out[ts(ic * 4 + isub, P), :], in_=o_sb)

    nc.compile()
    return nc


_NC_CACHE = None


def _get_module():
    global _NC_CACHE
    if _NC_CACHE is None:
        _NC_CACHE = build_module()
    return _NC_CACHE


def kernel(target, source_k, source_v, Wq, bq, Wk, bk, Wv, bv,
           g_t, b_t, g_k, b_k, g_v, b_v):
    target = np.asarray(target, dtype=np.float32)
    source_k = np.asarray(source_k, dtype=np.float32)
    source_v = np.asarray(source_v, dtype=np.float32)
    Wq = np.asarray(Wq, dtype=np.float32); bq = np.asarray(bq, dtype=np.float32)
    Wk = np.asarray(Wk, dtype=np.float32); bk = np.asarray(bk, dtype=np.float32)
    Wv = np.asarray(Wv, dtype=np.float32); bv = np.asarray(bv, dtype=np.float32)
    g_t = np.asarray(g_t, dtype=np.float32); b_t = np.asarray(b_t, dtype=np.float32)
    g_k = np.asarray(g_k, dtype=np.float32); b_k = np.asarray(b_k, dtype=np.float32)
    g_v = np.asarray(g_v, dtype=np.float32); b_v = np.asarray(b_v, dtype=np.float32)

    bf16 = ml_dtypes.bfloat16
    # Fold the layernorm affine (g, b) into the projection weights/biases:
    #   LN_affine(x) @ W.T + b  ==  LN_plain(x) @ (W*g).T + (b + W @ b_ln)
    wqT = np.ascontiguousarray((Wq * g_t[None, :]).T).astype(bf16)
    wkT = np.ascontiguousarray((Wk * g_k[None, :]).T).astype(bf16)
    wvT = np.ascontiguousarray((Wv * g_v[None, :]).T).astype(bf16)
    bq_f = bq + Wq @ b_t
    bk_f = bk + Wk @ b_k
    bv_f = bv + Wv @ b_v

    nc = _get_module()
    in_maps = []
    for b in range(B):
        in_maps.append({
            "x_t": np.ascontiguousarray(target[b]),
            "x_k": np.ascontiguousarray(source_k[b]),
            "x_v": np.ascontiguousarray(source_v[b]),
            "wq": wqT, "wk": wkT, "wv": wvT,
            "bq": bq_f, "bk": bk_f, "bv": bv_f,
        })

    res = run_bass_kernel_spmd(nc, in_maps, core_ids=list(range(B)),
                               trace=bool(int(os.environ.get("KERNEL_TRACE", "0"))))
    out = np.stack([res.results[b]["out"] for b in range(B)], axis=0)
    kernel.last_results = res
    return out



# revision 2
# speedup vs baseline: 1.0374x; 1.0374x over previous
"""Single-head cross-attention (layernorm + QKV proj + softmax(QK^T)V) on 8 NeuronCores.

Sharding: data-parallel over batch B=8, one batch element per core.

v2 design (vs the DMA-xbar-transpose baseline):
  * Host ships BOTH natural fp32 x (for stats) and pre-transposed bf16 x^T
    (for matmuls) — no on-device activation transposes at all.
  * Layernorm is folded into the projections:
      q^T[e,i] = rstd_i * (W^T x^T_raw + colsum_w ⊗ (-mu) + b ⊗ inv_rstd)[e,i]
    The two rank-1 corrections ride a single K=2 matmul appended to each
    PSUM accumulation group; the per-token rstd scale is applied at PSUM
    evacuation (DVE tensor_mul with a broadcast rstd row for q/k in
    transposed layout; ScalarE per-partition scale for v in natural layout).
  * Stats rows (free-dim layout) are produced by a tiny PE transpose of the
    per-chunk stats columns; rstd broadcast goes through a DRAM round-trip.
  * Scores K^T·Q run in fp8(e4m3) with DoubleRow perf mode (2 d-blocks per
    matmul) — rel err ~1.2e-2 (gate 2e-2), everything else bf16/fp32.
  * attn·V and the Z (softmax denominator) matmuls as in the baseline.
"""

import os
from contextlib import ExitStack

import numpy as np
import ml_dtypes

import concourse.bass as bass
import concourse.bacc as bacc
import concourse.mybir as mybir
import concourse.tile as tile
from concourse.bass import ts, ds
from concourse.bass_utils import run_bass_kernel_spmd

BF16 = mybir.dt.bfloat16
F32 = mybir.dt.float32
F8 = mybir.dt.float8e4

B, T, D = 8, 2048, 1024
EPS = 1e-5
SCALE = float(D) ** -0.5
P = 128
N_IT = T // P          # 16 token tiles of 128
N_DB = D // P          # 8 d-blocks of 128
N_EB = D // P          # 8 e-blocks of 128
N_IC = T // 512        # 4 token chunks of 512
N_EC = D // 512        # 2 e chunks of 512

AF = mybir.ActivationFunctionType


def build_module() -> bass.Bass:
    nc = bacc.Bacc("TRN2", target_bir_lowering=False)

    x_nat = {}
    x_tr = {}
    w_d = {}
    cs_d = {}
    b_d = {}
    for nm in ("t", "k", "v"):
        x_nat[nm] = nc.dram_tensor(f"x_{nm}", [T, D], F32, kind="ExternalInput")
        x_tr[nm] = nc.dram_tensor(f"xt_{nm}", [D, T], BF16, kind="ExternalInput")
        w_d[nm] = nc.dram_tensor(f"w_{nm}", [D, D], BF16, kind="ExternalInput")  # [d, e]
        cs_d[nm] = nc.dram_tensor(f"cs_{nm}", [D], F32, kind="ExternalInput")
        b_d[nm] = nc.dram_tensor(f"b_{nm}", [D], F32, kind="ExternalInput")
    ident_d = nc.dram_tensor("ident", [P, P], F32, kind="ExternalInput")
    out = nc.dram_tensor("out", [T, D], F32, kind="ExternalOutput")
    rstd_dram = {nm: nc.dram_tensor(f"rstd_d_{nm}", [T], F32, kind="Internal")
                 for nm in ("t", "k")}

    with tile.TileContext(nc) as tc, ExitStack() as ctx:
        const = ctx.enter_context(tc.tile_pool(name="const", bufs=1))
        qkv = ctx.enter_context(tc.tile_pool(name="qkv", bufs=1))
        mm_ps = ctx.enter_context(tc.tile_pool(name="mm_ps", bufs=4, space="PSUM"))

        # ---- constants ----
        eps_t = const.tile([P, 1], F32)
        nc.vector.memset(eps_t, EPS)
        ones_t = const.tile([P, 1], BF16)
        nc.vector.memset(ones_t, 1.0)
        ident = const.tile([P, P], F32)
        nc.sync.dma_start(out=ident, in_=ident_d[:, :])

        # corr_w[X]: partition0 = colsum(w_eff), partition1 = folded bias
        corr_w = {}
        for nm in ("t", "k", "v"):
            cw = qkv.tile([2, D], F32, name=f"corr_w_{nm}")
            nc.sync.dma_start(out=cw[ds(0, 1), :], in_=cs_d[nm][:].unsqueeze(0))
            nc.sync.dma_start(out=cw[ds(1, 1), :], in_=b_d[nm][:].unsqueeze(0))
            corr_w[nm] = cw

        # ---- persistent projection outputs ----
        qT = qkv.tile([P, N_EB, T], F8)          # qT[p, eb, i] = q[i, eb*128+p]
        kT = qkv.tile([P, N_EB, T], F8)
        v_sb = qkv.tile([P, N_IT, D], BF16)      # v[p, jt, e] = v[jt*128+p, e]
        corr_rows = {nm: qkv.tile([2, T], F32, name=f"corr_rows_{nm}")
                     for nm in ("t", "k", "v")}  # p0 = -mu, p1 = sqrt(var+eps)
        rstd_bc = {nm: qkv.tile([P, T], F32, name=f"rstd_bc_{nm}")
                   for nm in ("t", "k")}         # rstd broadcast along partitions

        with tc.tile_pool(name="stats_ps", bufs=2, space="PSUM") as stats_ps, \
             tc.tile_pool(name="proj_phase", bufs=1) as pp:

            def stats_chunk(nm, ic):
                """Stats for token chunk ic of tensor nm -> corr_rows / rstd."""
                scols = pp.tile([P, 12], F32, tag="scols", bufs=2,
                                name=f"scols_{nm}_{ic}")
                for tl in range(4):
                    it = 4 * ic + tl
                    x_raw = pp.tile([P, D], F32, tag="x_raw", bufs=3,
                                    name=f"x_raw_{nm}_{it}")
                    nc.gpsimd.dma_start(out=x_raw, in_=x_nat[nm][ts(it, P), :])
                    st6 = pp.tile([P, 2, 6], F32, tag="st6", bufs=3,
                                  name=f"st6_{nm}_{it}")
                    for sb in range(2):
                        nc.vector.bn_stats(out=st6[:, sb, :], in_=x_raw[:, ts(sb, 512)])
                    mv = pp.tile([P, 2], F32, tag="mv", bufs=3,
                                 name=f"mv_{nm}_{it}")
                    nc.vector.bn_aggr(out=mv, in_=st6)
                    # col 0..3: -mu ; col 4..7: sqrt(var+eps) ; col 8..11: rstd
                    nc.scalar.activation(out=scols[:, tl:tl + 1], in_=mv[:, 0:1],
                                         func=AF.Copy, scale=-1.0)
                    nc.scalar.activation(out=scols[:, 4 + tl:5 + tl], in_=mv[:, 1:2],
                                         func=AF.Sqrt, bias=eps_t)
                    nc.vector.reciprocal(out=scols[:, 8 + tl:9 + tl],
                                         in_=scols[:, 4 + tl:5 + tl])
                st_ps = stats_ps.tile([12, P], F32, tag="stats",
                                      name=f"stps_{nm}_{ic}")
                nc.tensor.transpose(st_ps, in_=scols, identity=ident)
                st_sb = pp.tile([12, P], F32, tag="st_sb", bufs=2,
                                name=f"stsb_{nm}_{ic}")
                nc.vector.tensor_copy(out=st_sb, in_=st_ps)
                nc.sync.dma_start(out=corr_rows[nm][ds(0, 1), ts(ic, 512)],
                                  in_=st_sb[ds(0, 4), :])
                nc.sync.dma_start(out=corr_rows[nm][ds(1, 1), ts(ic, 512)],
                                  in_=st_sb[ds(4, 4), :])
                if nm in ("t", "k"):
                    nc.sync.dma_start(out=rstd_dram[nm][ts(ic, 512)],
                                      in_=st_sb[ds(8, 4), :])
                    rd = rstd_dram[nm][ts(ic, 512)]
                    nc.sync.dma_start(
                        out=rstd_bc[nm][:, ts(ic, 512)],
                        in_=bass.AP(tensor=rd.tensor, offset=rd.offset,
                                    ap=[[0, P]] + list(rd.ap)))
                return scols

            def load_w(nm):
                w_all = pp.tile([P, N_DB, D], BF16, tag="w_all", bufs=2,
                                name=f"w_all_{nm}")
                nc.sync.dma_start(
                    out=w_all, in_=w_d[nm][:, :].rearrange("(a p) e -> p a e", p=P))
                return w_all

            def load_xt(nm, ic):
                xt_sb = pp.tile([P, N_DB, 512], BF16, tag="xt", bufs=3,
                                name=f"xt_{nm}_{ic}")
                nc.sync.dma_start(
                    out=xt_sb,
                    in_=x_tr[nm][:, ts(ic, 512)].rearrange("(a p) t -> p a t", p=P))
                return xt_sb

            def proj_qk(nm, dstT, w_all, xt_sb, ic):
                """dstT[:, eb, chunk ic] in fp8, LN+bias folded."""
                for eb in range(N_EB):
                    ps = mm_ps.tile([P, 512], F32, tag="mm",
                                    name=f"ps_{nm}_{ic}_{eb}")
                    for db in range(N_DB):
                        nc.tensor.matmul(ps, lhsT=w_all[:, db, ts(eb, P)],
                                         rhs=xt_sb[:, db, :],
                                         start=(db == 0), stop=False)
                    nc.tensor.matmul(ps, lhsT=corr_w[nm][:, ts(eb, P)],
                                     rhs=corr_rows[nm][:, ts(ic, 512)],
                                     start=False, stop=True)
                    nc.vector.tensor_mul(out=dstT[:, eb, ts(ic, 512)], in0=ps,
                                         in1=rstd_bc[nm][:, ts(ic, 512)])

            def proj_v(w_all, xt_sb, ic, scols):
                for ec in range(N_EC):
                    for ml in range(4):
                        m = 4 * ic + ml
                        ps = mm_ps.tile([P, 512], F32, tag="mm",
                                        name=f"ps_v_{m}_{ec}")
                        for db in range(N_DB):
                            nc.tensor.matmul(ps,
                                             lhsT=xt_sb[:, db, ds(ml * P, P)],
                                             rhs=w_all[:, db, ts(ec, 512)],
                                             start=(db == 0), stop=False)
                        nc.tensor.matmul(ps, lhsT=corr_rows["v"][:, ts(m, P)],
                                         rhs=corr_w["v"][:, ts(ec, 512)],
                                         start=False, stop=True)
                        nc.scalar.activation(out=v_sb[:, m, ts(ec, 512)], in_=ps,
                                             func=AF.Identity,
                                             scale=scols[:, 8 + ml:9 + ml])

            # ---- source_k -> kT ----
            w_k = load_w("k")
            with nc.named_scope("proj_k"):
                for ic in range(N_IC):
                    stats_chunk("k", ic)
                    xt_sb = load_xt("k", ic)
                    proj_qk("k", kT, w_k, xt_sb, ic)
            # ---- target -> qT ----
            w_q = load_w("t")
            with nc.named_scope("proj_q"):
                for ic in range(N_IC):
                    stats_chunk("t", ic)
                    xt_sb = load_xt("t", ic)
                    proj_qk("t", qT, w_q, xt_sb, ic)
            # ---- source_v -> v ----
            w_v = load_w("v")
            with nc.named_scope("proj_v"):
                for ic in range(N_IC):
                    scols = stats_chunk("v", ic)
                    xt_sb = load_xt("v", ic)
                    proj_v(w_v, xt_sb, ic, scols)

        # ---- attention ----
        attv_ps = ctx.enter_context(tc.tile_pool(name="attv_ps", bufs=2, space="PSUM"))
        with tc.tile_pool(name="att", bufs=1) as att:
            for ic in range(N_IC):
                with nc.named_scope(f"scores_{ic}"):
                    aT = att.tile([P, N_IT, 512], BF16, tag="aT", bufs=2,
                                  name=f"aT_{ic}")
                    for jt in range(N_IT):
                        ps = mm_ps.tile([P, 512], F32, tag="mm",
                                        name=f"ps_s_{ic}_{jt}")
                        for ebp in range(N_EB // 2):
                            nc.tensor.matmul(
                                ps, lhsT=kT[:, ds(2 * ebp, 2), ts(jt, P)],
                                rhs=qT[:, ds(2 * ebp, 2), ts(ic, 512)],
                                start=(ebp == 0), stop=(ebp == N_EB // 2 - 1),
                                perf_mode=mybir.MatmulPerfMode.DoubleRow)
                        nc.scalar.activation(
                            out=aT[:, jt, :], in_=ps,
                            func=AF.Exp, scale=SCALE)
                with nc.named_scope(f"attv_{ic}"):
                    for isub in range(4):
                        ou = attv_ps.tile([P, D], F32, tag="ou",
                                          name=f"ou_{ic}_{isub}")
                        zz = mm_ps.tile([P, 1], F32, tag="mm",
                                        name=f"z_{ic}_{isub}")
                        # same-bank runs of 16 accumulating matmuls (bank cycling
                        # between consecutive matmuls forces PE micro-stalls)
                        for ec in range(N_EC):
                            for jt in range(N_IT):
                                nc.tensor.matmul(
                                    ou[:, ts(ec, 512)],
                                    lhsT=aT[:, jt, ts(isub, P)],
                                    rhs=v_sb[:, jt, ts(ec, 512)],
                                    start=(jt == 0), stop=(jt == N_IT - 1))
                        for jt in range(N_IT):
                            nc.tensor.matmul(zz, lhsT=aT[:, jt, ts(isub, P)],
                                             rhs=ones_t,
                                             start=(jt == 0), stop=(jt == N_IT - 1))
                        rz = att.tile([P, 1], F32, tag="rz", bufs=2,
                                      name=f"rz_{ic}_{isub}")
                        nc.vector.reciprocal(out=rz, in_=zz)
                        o_sb = att.tile([P, D], F32, tag="o_sb", bufs=2,
                                        name=f"o_{ic}_{isub}")
                        nc.vector.tensor_scalar_mul(out=o_sb, in0=ou, scalar1=rz)
                        nc.sync.dma_start(out=out[ts(ic * 4 + isub, P), :], in_=o_sb)

    nc.compile()
    return nc


_NC_CACHE = None


def _get_module():
    global _NC_CACHE
    if _NC_CACHE is None:
        _NC_CACHE = build_module()
    return _NC_CACHE


def host_prep(target, source_k, source_v, Wq, bq, Wk, bk, Wv, bv,
              g_t, b_t, g_k, b_k, g_v, b_v):
    """Shared host-side input prep; returns per-core in_maps."""
    bf16 = ml_dtypes.bfloat16
    f32 = np.float32
    Wq = np.asarray(Wq, f32); bq = np.asarray(bq, f32)
    Wk = np.asarray(Wk, f32); bk = np.asarray(bk, f32)
    Wv = np.asarray(Wv, f32); bv = np.asarray(bv, f32)
    g_t = np.asarray(g_t, f32); b_t = np.asarray(b_t, f32)
    g_k = np.asarray(g_k, f32); b_k = np.asarray(b_k, f32)
    g_v = np.asarray(g_v, f32); b_v = np.asarray(b_v, f32)

    # Fold the layernorm affine (g, b) into the projection weights/biases:
    #   LN_affine(x) @ W.T + b  ==  LN_plain(x) @ (W*g).T + (b + W @ b_ln)
    wts = {"t": np.ascontiguousarray((Wq * g_t[None, :]).T).astype(bf16),
           "k": np.ascontiguousarray((Wk * g_k[None, :]).T).astype(bf16),
           "v": np.ascontiguousarray((Wv * g_v[None, :]).T).astype(bf16)}
    bias = {"t": bq + Wq @ b_t, "k": bk + Wk @ b_k, "v": bv + Wv @ b_v}
    csum = {nm: wts[nm].astype(f32).sum(axis=0) for nm in wts}
    ident = np.eye(P, dtype=f32)

    xs = {"t": np.asarray(target, f32), "k": np.asarray(source_k, f32),
          "v": np.asarray(source_v, f32)}
    in_maps = []
    for b in range(B):
        im = {"ident": ident}
        for nm in ("t", "k", "v"):
            im[f"x_{nm}"] = np.ascontiguousarray(xs[nm][b])
            im[f"xt_{nm}"] = np.ascontiguousarray(xs[nm][b].T).astype(bf16)
            im[f"w_{nm}"] = wts[nm]
            im[f"cs_{nm}"] = csum[nm]
            im[f"b_{nm}"] = bias[nm]
        in_maps.append(im)
    return in_maps


def kernel(target, source_k, source_v, Wq, bq, Wk, bk, Wv, bv,
           g_t, b_t, g_k, b_k, g_v, b_v):
    in_maps = host_prep(target, source_k, source_v, Wq, bq, Wk, bk, Wv, bv,
                        g_t, b_t, g_k, b_k, g_v, b_v)
    nc = _get_module()
    res = run_bass_kernel_spmd(nc, in_maps, core_ids=list(range(B)),
                               trace=bool(int(os.environ.get("KERNEL_TRACE", "0"))))
    out = np.stack([res.results[b]["out"] for b in range(B)], axis=0)
    kernel.last_results = res
    return out


# revision 3
# speedup vs baseline: 1.4133x; 1.3624x over previous
"""Single-head cross-attention (layernorm + QKV proj + softmax(QK^T)V) on 8 NeuronCores.

Sharding: data-parallel over batch B=8, one batch element per core.

v2 design (vs the DMA-xbar-transpose baseline):
  * Host ships BOTH natural fp32 x (for stats) and pre-transposed bf16 x^T
    (for matmuls) — no on-device activation transposes at all.
  * Layernorm is folded into the projections:
      q^T[e,i] = rstd_i * (W^T x^T_raw + colsum_w ⊗ (-mu) + b ⊗ inv_rstd)[e,i]
    The two rank-1 corrections ride a single K=2 matmul appended to each
    PSUM accumulation group; the per-token rstd scale is applied at PSUM
    evacuation (DVE tensor_mul with a broadcast rstd row for q/k in
    transposed layout; ScalarE per-partition scale for v in natural layout).
  * Stats rows (free-dim layout) are produced by a tiny PE transpose of the
    per-chunk stats columns; rstd broadcast goes through a DRAM round-trip.
  * Scores K^T·Q run in fp8(e4m3) with DoubleRow perf mode (2 d-blocks per
    matmul) — rel err ~1.2e-2 (gate 2e-2), everything else bf16/fp32.
  * attn·V and the Z (softmax denominator) matmuls as in the baseline.
"""

import os
from contextlib import ExitStack

import numpy as np
import ml_dtypes

import concourse.bass as bass
import concourse.bacc as bacc
import concourse.mybir as mybir
import concourse.tile as tile
from concourse.bass import ts, ds
from concourse.bass_utils import run_bass_kernel_spmd

BF16 = mybir.dt.bfloat16
F32 = mybir.dt.float32
F8 = mybir.dt.float8e4

B, T, D = 8, 2048, 1024
EPS = 1e-5
SCALE = float(D) ** -0.5
P = 128
N_IT = T // P          # 16 token tiles of 128
N_DB = D // P          # 8 d-blocks of 128
N_EB = D // P          # 8 e-blocks of 128
N_IC = T // 512        # 4 token chunks of 512
N_EC = D // 512        # 2 e chunks of 512

AF = mybir.ActivationFunctionType


def build_module() -> bass.Bass:
    nc = bacc.Bacc("TRN2", target_bir_lowering=False)

    x_nat = {}
    x_tr = {}
    w_d = {}
    cs_d = {}
    b_d = {}
    for nm in ("t", "k", "v"):
        x_nat[nm] = nc.dram_tensor(f"x_{nm}", [T, D], F32, kind="ExternalInput")
        x_tr[nm] = nc.dram_tensor(f"xt_{nm}", [D, T], BF16, kind="ExternalInput")
        w_d[nm] = nc.dram_tensor(f"w_{nm}", [D, D], BF16, kind="ExternalInput")  # [d, e]
        cs_d[nm] = nc.dram_tensor(f"cs_{nm}", [D], BF16, kind="ExternalInput")
        b_d[nm] = nc.dram_tensor(f"b_{nm}", [D], BF16, kind="ExternalInput")
    ident_d = nc.dram_tensor("ident", [P, P], F32, kind="ExternalInput")
    out = nc.dram_tensor("out", [T, D], F32, kind="ExternalOutput")
    rstd_dram = {nm: nc.dram_tensor(f"rstd_d_{nm}", [T], BF16, kind="Internal")
                 for nm in ("t", "k")}

    with tile.TileContext(nc) as tc, ExitStack() as ctx:
        const = ctx.enter_context(tc.tile_pool(name="const", bufs=1))
        qkv = ctx.enter_context(tc.tile_pool(name="qkv", bufs=1))
        mm_ps = ctx.enter_context(tc.tile_pool(name="mm_ps", bufs=4, space="PSUM"))

        # ---- constants ----
        eps_t = const.tile([P, 1], F32)
        nc.vector.memset(eps_t, EPS)
        ones_t = const.tile([P, 1], BF16)
        nc.vector.memset(ones_t, 1.0)
        ident = const.tile([P, P], F32)
        nc.sync.dma_start(out=ident, in_=ident_d[:, :])

        # corr_w[X]: partition0 = colsum(w_eff), partition1 = folded bias.
        # bf16: fp32 matmuls run at 1/4 rate (2 half-speed passes) and the
        # correction terms are small relative to q/k/v (~3% magnitude).
        corr_w = {}
        for nm in ("t", "k", "v"):
            cw = qkv.tile([2, D], BF16, name=f"corr_w_{nm}")
            nc.sync.dma_start(out=cw[ds(0, 1), :], in_=cs_d[nm][:].unsqueeze(0))
            nc.sync.dma_start(out=cw[ds(1, 1), :], in_=b_d[nm][:].unsqueeze(0))
            corr_w[nm] = cw

        # ---- persistent projection outputs ----
        qT = qkv.tile([P, N_EB, T], F8)          # qT[p, eb, i] = q[i, eb*128+p]
        kT = qkv.tile([P, N_EB, T], F8)
        v_sb = qkv.tile([P, N_IT, D], BF16)      # v[p, jt, e] = v[jt*128+p, e]
        corr_rows = {nm: qkv.tile([2, T], BF16, name=f"corr_rows_{nm}")
                     for nm in ("t", "k", "v")}  # p0 = -mu, p1 = sqrt(var+eps)
        rstd_bc = {nm: qkv.tile([P, T], BF16, name=f"rstd_bc_{nm}")
                   for nm in ("t", "k")}         # rstd broadcast along partitions
        rstd_cols_v = qkv.tile([P, N_IT], F32)   # v rstd, natural col layout

        with tc.tile_pool(name="stats_ps", bufs=2, space="PSUM") as stats_ps, \
             tc.tile_pool(name="proj_phase", bufs=1) as pp:

            def stats_chunk(nm, ic):
                """Stats for token chunk ic of tensor nm -> corr_rows / rstd."""
                ncol = 12 if nm in ("t", "k") else 8
                scols = pp.tile([P, 12], F32, tag="scols", bufs=2,
                                name=f"scols_{nm}_{ic}")
                for tl in range(4):
                    it = 4 * ic + tl
                    x_raw = pp.tile([P, D], F32, tag="x_raw", bufs=3,
                                    name=f"x_raw_{nm}_{it}")
                    nc.gpsimd.dma_start(out=x_raw, in_=x_nat[nm][ts(it, P), :])
                    st6 = pp.tile([P, 2, 6], F32, tag="st6", bufs=3,
                                  name=f"st6_{nm}_{it}")
                    for sb in range(2):
                        nc.vector.bn_stats(out=st6[:, sb, :], in_=x_raw[:, ts(sb, 512)])
                    mv = pp.tile([P, 2], F32, tag="mv", bufs=3,
                                 name=f"mv_{nm}_{it}")
                    nc.vector.bn_aggr(out=mv, in_=st6)
                    # col 0..3: -mu ; col 4..7: sqrt(var+eps) ; col 8..11: rstd
                    nc.scalar.activation(out=scols[:, tl:tl + 1], in_=mv[:, 0:1],
                                         func=AF.Copy, scale=-1.0)
                    nc.scalar.activation(out=scols[:, 4 + tl:5 + tl], in_=mv[:, 1:2],
                                         func=AF.Sqrt, bias=eps_t)
                    rstd_out = (rstd_cols_v[:, it:it + 1] if nm == "v"
                                else scols[:, 8 + tl:9 + tl])
                    nc.vector.reciprocal(out=rstd_out,
                                         in_=scols[:, 4 + tl:5 + tl])
                st_ps = stats_ps.tile([12, P], F32, tag="stats",
                                      name=f"stps_{nm}_{ic}")
                nc.tensor.transpose(st_ps[ds(0, ncol), :], in_=scols[:, 0:ncol],
                                    identity=ident)
                st_sb = pp.tile([12, P], BF16, tag="st_sb", bufs=2,
                                name=f"stsb_{nm}_{ic}")
                nc.vector.tensor_copy(out=st_sb[ds(0, ncol), :],
                                      in_=st_ps[ds(0, ncol), :])
                nc.sync.dma_start(out=corr_rows[nm][ds(0, 1), ts(ic, 512)],
                                  in_=st_sb[ds(0, 4), :])
                nc.sync.dma_start(out=corr_rows[nm][ds(1, 1), ts(ic, 512)],
                                  in_=st_sb[ds(4, 4), :])
                if nm in ("t", "k"):
                    nc.sync.dma_start(out=rstd_dram[nm][ts(ic, 512)],
                                      in_=st_sb[ds(8, 4), :])
                    rd = rstd_dram[nm][ts(ic, 512)]
                    nc.sync.dma_start(
                        out=rstd_bc[nm][:, ts(ic, 512)],
                        in_=bass.AP(tensor=rd.tensor, offset=rd.offset,
                                    ap=[[0, P]] + list(rd.ap)))

            def load_w(nm):
                w_all = pp.tile([P, N_DB, D], BF16, tag="w_all", bufs=2,
                                name=f"w_all_{nm}")
                nc.sync.dma_start(
                    out=w_all, in_=w_d[nm][:, :].rearrange("(a p) e -> p a e", p=P))
                return w_all

            def load_xt(nm, ic):
                xt_sb = pp.tile([P, N_DB, 512], BF16, tag="xt", bufs=3,
                                name=f"xt_{nm}_{ic}")
                nc.sync.dma_start(
                    out=xt_sb,
                    in_=x_tr[nm][:, ts(ic, 512)].rearrange("(a p) t -> p a t", p=P))
                return xt_sb

            def proj_qk(nm, dstT, w_all, xt_sb, ic):
                """dstT[:, eb, chunk ic] in fp8, LN+bias folded."""
                for eb in range(N_EB):
                    ps = mm_ps.tile([P, 512], F32, tag="mm",
                                    name=f"ps_{nm}_{ic}_{eb}")
                    for db in range(N_DB):
                        nc.tensor.matmul(ps, lhsT=w_all[:, db, ts(eb, P)],
                                         rhs=xt_sb[:, db, :],
                                         start=(db == 0), stop=False)
                    nc.tensor.matmul(ps, lhsT=corr_w[nm][:, ts(eb, P)],
                                     rhs=corr_rows[nm][:, ts(ic, 512)],
                                     start=False, stop=True)
                    nc.vector.tensor_mul(out=dstT[:, eb, ts(ic, 512)], in0=ps,
                                         in1=rstd_bc[nm][:, ts(ic, 512)])

            def proj_v(w_all, xt_sb, ic):
                for ec in range(N_EC):
                    for ml in range(4):
                        m = 4 * ic + ml
                        ps = mm_ps.tile([P, 512], F32, tag="mm",
                                        name=f"ps_v_{m}_{ec}")
                        for db in range(N_DB):
                            nc.tensor.matmul(ps,
                                             lhsT=xt_sb[:, db, ds(ml * P, P)],
                                             rhs=w_all[:, db, ts(ec, 512)],
                                             start=(db == 0), stop=False)
                        nc.tensor.matmul(ps, lhsT=corr_rows["v"][:, ts(m, P)],
                                         rhs=corr_w["v"][:, ts(ec, 512)],
                                         start=False, stop=True)
                        nc.scalar.activation(out=v_sb[:, m, ts(ec, 512)], in_=ps,
                                             func=AF.Identity,
                                             scale=rstd_cols_v[:, m:m + 1])

            # Each tensor's stats chain is emitted one projection-phase early
            # so the in-order DVE/PE queues have it ready when the projection
            # needs corr rows (avoids a stall at each phase transition).
            w_k = load_w("k")
            with nc.named_scope("proj_k"):
                for ic in range(N_IC):
                    stats_chunk("k", ic)
                    xt_sb = load_xt("k", ic)
                    proj_qk("k", kT, w_k, xt_sb, ic)
                    stats_chunk("t", ic)
            w_q = load_w("t")
            with nc.named_scope("proj_q"):
                for ic in range(N_IC):
                    xt_sb = load_xt("t", ic)
                    proj_qk("t", qT, w_q, xt_sb, ic)
                    stats_chunk("v", ic)
            w_v = load_w("v")
            with nc.named_scope("proj_v"):
                for ic in range(N_IC):
                    xt_sb = load_xt("v", ic)
                    proj_v(w_v, xt_sb, ic)

        # ---- attention ----
        attv_ps = ctx.enter_context(tc.tile_pool(name="attv_ps", bufs=2, space="PSUM"))
        with tc.tile_pool(name="att", bufs=1) as att:
            for ic in range(N_IC):
                with nc.named_scope(f"scores_{ic}"):
                    aT = att.tile([P, N_IT, 512], BF16, tag="aT", bufs=2,
                                  name=f"aT_{ic}")
                    for jt in range(N_IT):
                        ps = mm_ps.tile([P, 512], F32, tag="mm",
                                        name=f"ps_s_{ic}_{jt}")
                        for ebp in range(N_EB // 2):
                            nc.tensor.matmul(
                                ps, lhsT=kT[:, ds(2 * ebp, 2), ts(jt, P)],
                                rhs=qT[:, ds(2 * ebp, 2), ts(ic, 512)],
                                start=(ebp == 0), stop=(ebp == N_EB // 2 - 1),
                                perf_mode=mybir.MatmulPerfMode.DoubleRow)
                        nc.scalar.activation(
                            out=aT[:, jt, :], in_=ps,
                            func=AF.Exp, scale=SCALE)
                with nc.named_scope(f"attv_{ic}"):
                    for isub in range(4):
                        ou = attv_ps.tile([P, D], F32, tag="ou",
                                          name=f"ou_{ic}_{isub}")
                        zz = mm_ps.tile([P, 1], F32, tag="mm",
                                        name=f"z_{ic}_{isub}")
                        # same-bank runs of 16 accumulating matmuls (bank cycling
                        # between consecutive matmuls forces PE micro-stalls)
                        for ec in range(N_EC):
                            for jt in range(N_IT):
                                nc.tensor.matmul(
                                    ou[:, ts(ec, 512)],
                                    lhsT=aT[:, jt, ts(isub, P)],
                                    rhs=v_sb[:, jt, ts(ec, 512)],
                                    start=(jt == 0), stop=(jt == N_IT - 1))
                        for jt in range(N_IT):
                            nc.tensor.matmul(zz, lhsT=aT[:, jt, ts(isub, P)],
                                             rhs=ones_t,
                                             start=(jt == 0), stop=(jt == N_IT - 1))
                        rz = att.tile([P, 1], F32, tag="rz", bufs=2,
                                      name=f"rz_{ic}_{isub}")
                        nc.vector.reciprocal(out=rz, in_=zz)
                        o_sb = att.tile([P, D], F32, tag="o_sb", bufs=2,
                                        name=f"o_{ic}_{isub}")
                        nc.vector.tensor_scalar_mul(out=o_sb, in0=ou, scalar1=rz)
                        nc.sync.dma_start(out=out[ts(ic * 4 + isub, P), :], in_=o_sb)

    nc.compile()
    return nc


_NC_CACHE = None


def _get_module():
    global _NC_CACHE
    if _NC_CACHE is None:
        _NC_CACHE = build_module()
    return _NC_CACHE


def host_prep(target, source_k, source_v, Wq, bq, Wk, bk, Wv, bv,
              g_t, b_t, g_k, b_k, g_v, b_v):
    """Shared host-side input prep; returns per-core in_maps."""
    bf16 = ml_dtypes.bfloat16
    f32 = np.float32
    Wq = np.asarray(Wq, f32); bq = np.asarray(bq, f32)
    Wk = np.asarray(Wk, f32); bk = np.asarray(bk, f32)
    Wv = np.asarray(Wv, f32); bv = np.asarray(bv, f32)
    g_t = np.asarray(g_t, f32); b_t = np.asarray(b_t, f32)
    g_k = np.asarray(g_k, f32); b_k = np.asarray(b_k, f32)
    g_v = np.asarray(g_v, f32); b_v = np.asarray(b_v, f32)

    # Fold the layernorm affine (g, b) into the projection weights/biases:
    #   LN_affine(x) @ W.T + b  ==  LN_plain(x) @ (W*g).T + (b + W @ b_ln)
    wts = {"t": np.ascontiguousarray((Wq * g_t[None, :]).T).astype(bf16),
           "k": np.ascontiguousarray((Wk * g_k[None, :]).T).astype(bf16),
           "v": np.ascontiguousarray((Wv * g_v[None, :]).T).astype(bf16)}
    bias = {"t": (bq + Wq @ b_t).astype(bf16), "k": (bk + Wk @ b_k).astype(bf16),
            "v": (bv + Wv @ b_v).astype(bf16)}
    csum = {nm: wts[nm].astype(f32).sum(axis=0).astype(bf16) for nm in wts}
    ident = np.eye(P, dtype=f32)

    xs = {"t": np.asarray(target, f32), "k": np.asarray(source_k, f32),
          "v": np.asarray(source_v, f32)}
    in_maps = []
    for b in range(B):
        im = {"ident": ident}
        for nm in ("t", "k", "v"):
            im[f"x_{nm}"] = np.ascontiguousarray(xs[nm][b])
            im[f"xt_{nm}"] = np.ascontiguousarray(xs[nm][b].T).astype(bf16)
            im[f"w_{nm}"] = wts[nm]
            im[f"cs_{nm}"] = csum[nm]
            im[f"b_{nm}"] = bias[nm]
        in_maps.append(im)
    return in_maps


def kernel(target, source_k, source_v, Wq, bq, Wk, bk, Wv, bv,
           g_t, b_t, g_k, b_k, g_v, b_v):
    in_maps = host_prep(target, source_k, source_v, Wq, bq, Wk, bk, Wv, bv,
                        g_t, b_t, g_k, b_k, g_v, b_v)
    nc = _get_module()
    res = run_bass_kernel_spmd(nc, in_maps, core_ids=list(range(B)),
                               trace=bool(int(os.environ.get("KERNEL_TRACE", "0"))))
    out = np.stack([res.results[b]["out"] for b in range(B)], axis=0)
    kernel.last_results = res
    return out


# revision 4
# speedup vs baseline: 1.4619x; 1.0343x over previous
"""Single-head cross-attention (layernorm + QKV proj + softmax(QK^T)V) on 8 NeuronCores.

Sharding: data-parallel over batch B=8, one batch element per core.

v2 design (vs the DMA-xbar-transpose baseline):
  * Host ships BOTH natural fp32 x (for stats) and pre-transposed bf16 x^T
    (for matmuls) — no on-device activation transposes at all.
  * Layernorm is folded into the projections:
      q^T[e,i] = rstd_i * (W^T x^T_raw + colsum_w ⊗ (-mu) + b ⊗ inv_rstd)[e,i]
    The two rank-1 corrections ride a single K=2 matmul appended to each
    PSUM accumulation group; the per-token rstd scale is applied at PSUM
    evacuation (DVE tensor_mul with a broadcast rstd row for q/k in
    transposed layout; ScalarE per-partition scale for v in natural layout).
  * Stats rows (free-dim layout) are produced by a tiny PE transpose of the
    per-chunk stats columns; rstd broadcast goes through a DRAM round-trip.
  * Scores K^T·Q run in fp8(e4m3) with DoubleRow perf mode (2 d-blocks per
    matmul) — rel err ~1.2e-2 (gate 2e-2), everything else bf16/fp32.
  * attn·V and the Z (softmax denominator) matmuls as in the baseline.
"""

import os
from contextlib import ExitStack

import numpy as np
import ml_dtypes

import concourse.bass as bass
import concourse.bacc as bacc
import concourse.mybir as mybir
import concourse.tile as tile
from concourse.bass import ts, ds
from concourse.bass_utils import run_bass_kernel_spmd

BF16 = mybir.dt.bfloat16
F32 = mybir.dt.float32
F8 = mybir.dt.float8e4

B, T, D = 8, 2048, 1024
EPS = 1e-5
SCALE = float(D) ** -0.5
P = 128
N_IT = T // P          # 16 token tiles of 128
N_DB = D // P          # 8 d-blocks of 128
N_EB = D // P          # 8 e-blocks of 128
N_IC = T // 512        # 4 token chunks of 512
N_EC = D // 512        # 2 e chunks of 512

AF = mybir.ActivationFunctionType


def build_module() -> bass.Bass:
    nc = bacc.Bacc("TRN2", target_bir_lowering=False)

    x_nat = {}
    x_tr = {}
    w_d = {}
    cs_d = {}
    b_d = {}
    for nm in ("t", "k", "v"):
        x_nat[nm] = nc.dram_tensor(f"x_{nm}", [T, D], BF16, kind="ExternalInput")
        x_tr[nm] = nc.dram_tensor(f"xt_{nm}", [D, T], BF16, kind="ExternalInput")
        w_d[nm] = nc.dram_tensor(f"w_{nm}", [D, D], BF16, kind="ExternalInput")  # [d, e]
        cs_d[nm] = nc.dram_tensor(f"cs_{nm}", [D], BF16, kind="ExternalInput")
        b_d[nm] = nc.dram_tensor(f"b_{nm}", [D], BF16, kind="ExternalInput")
    ident_d = nc.dram_tensor("ident", [P, P], F32, kind="ExternalInput")
    out = nc.dram_tensor("out", [T, D], F32, kind="ExternalOutput")
    rstd_dram = {nm: nc.dram_tensor(f"rstd_d_{nm}", [T], BF16, kind="Internal")
                 for nm in ("t", "k")}

    with tile.TileContext(nc) as tc, ExitStack() as ctx:
        const = ctx.enter_context(tc.tile_pool(name="const", bufs=1))
        qkv = ctx.enter_context(tc.tile_pool(name="qkv", bufs=1))
        mm_ps = ctx.enter_context(tc.tile_pool(name="mm_ps", bufs=4, space="PSUM"))

        # ---- constants ----
        eps_t = const.tile([P, 1], F32)
        nc.vector.memset(eps_t, EPS)
        ones_t = const.tile([P, 1], BF16)
        nc.vector.memset(ones_t, 1.0)
        ident = const.tile([P, P], F32)
        nc.sync.dma_start(out=ident, in_=ident_d[:, :])

        # corr_w[X]: partition0 = colsum(w_eff), partition1 = folded bias.
        # bf16: fp32 matmuls run at 1/4 rate (2 half-speed passes) and the
        # correction terms are small relative to q/k/v (~3% magnitude).
        corr_w = {}
        for nm in ("t", "k", "v"):
            cw = qkv.tile([2, D], BF16, name=f"corr_w_{nm}")
            nc.sync.dma_start(out=cw[ds(0, 1), :], in_=cs_d[nm][:].unsqueeze(0))
            nc.sync.dma_start(out=cw[ds(1, 1), :], in_=b_d[nm][:].unsqueeze(0))
            corr_w[nm] = cw

        # ---- persistent projection outputs ----
        qT = qkv.tile([P, N_EB, T], F8)          # qT[p, eb, i] = q[i, eb*128+p]
        kT = qkv.tile([P, N_EB, T], F8)
        v_sb = qkv.tile([P, N_IT, D], BF16)      # v[p, jt, e] = v[jt*128+p, e]
        corr_rows = {nm: qkv.tile([2, T], BF16, name=f"corr_rows_{nm}")
                     for nm in ("t", "k", "v")}  # p0 = -mu, p1 = sqrt(var+eps)
        rstd_bc = {nm: qkv.tile([P, T], BF16, name=f"rstd_bc_{nm}")
                   for nm in ("t", "k")}         # rstd broadcast along partitions
        rstd_cols_v = qkv.tile([P, N_IT], F32)   # v rstd, natural col layout

        with tc.tile_pool(name="stats_ps", bufs=2, space="PSUM") as stats_ps, \
             tc.tile_pool(name="proj_phase", bufs=1) as pp:

            def stats_chunk(nm, ic):
                """Stats for token chunk ic of tensor nm -> corr_rows / rstd."""
                ncol = 12 if nm in ("t", "k") else 8
                scols = pp.tile([P, 12], F32, tag="scols", bufs=2,
                                name=f"scols_{nm}_{ic}")
                for tl in range(4):
                    it = 4 * ic + tl
                    x_raw = pp.tile([P, D], BF16, tag="x_raw", bufs=3,
                                    name=f"x_raw_{nm}_{it}")
                    nc.gpsimd.dma_start(out=x_raw, in_=x_nat[nm][ts(it, P), :])
                    st6 = pp.tile([P, 2, 6], F32, tag="st6", bufs=3,
                                  name=f"st6_{nm}_{it}")
                    for sb in range(2):
                        nc.vector.bn_stats(out=st6[:, sb, :], in_=x_raw[:, ts(sb, 512)])
                    mv = pp.tile([P, 2], F32, tag="mv", bufs=3,
                                 name=f"mv_{nm}_{it}")
                    nc.vector.bn_aggr(out=mv, in_=st6)
                    # col 0..3: -mu ; col 4..7: sqrt(var+eps) ; col 8..11: rstd
                    nc.scalar.activation(out=scols[:, tl:tl + 1], in_=mv[:, 0:1],
                                         func=AF.Copy, scale=-1.0)
                    nc.scalar.activation(out=scols[:, 4 + tl:5 + tl], in_=mv[:, 1:2],
                                         func=AF.Sqrt, bias=eps_t)
                    rstd_out = (rstd_cols_v[:, it:it + 1] if nm == "v"
                                else scols[:, 8 + tl:9 + tl])
                    nc.vector.reciprocal(out=rstd_out,
                                         in_=scols[:, 4 + tl:5 + tl])
                st_ps = stats_ps.tile([12, P], F32, tag="stats",
                                      name=f"stps_{nm}_{ic}")
                nc.tensor.transpose(st_ps[ds(0, ncol), :], in_=scols[:, 0:ncol],
                                    identity=ident)
                st_sb = pp.tile([12, P], BF16, tag="st_sb", bufs=2,
                                name=f"stsb_{nm}_{ic}")
                nc.vector.tensor_copy(out=st_sb[ds(0, ncol), :],
                                      in_=st_ps[ds(0, ncol), :])
                nc.sync.dma_start(out=corr_rows[nm][ds(0, 1), ts(ic, 512)],
                                  in_=st_sb[ds(0, 4), :])
                nc.sync.dma_start(out=corr_rows[nm][ds(1, 1), ts(ic, 512)],
                                  in_=st_sb[ds(4, 4), :])
                if nm in ("t", "k"):
                    nc.sync.dma_start(out=rstd_dram[nm][ts(ic, 512)],
                                      in_=st_sb[ds(8, 4), :])
                    rd = rstd_dram[nm][ts(ic, 512)]
                    nc.sync.dma_start(
                        out=rstd_bc[nm][:, ts(ic, 512)],
                        in_=bass.AP(tensor=rd.tensor, offset=rd.offset,
                                    ap=[[0, P]] + list(rd.ap)))

            def load_w(nm):
                w_all = pp.tile([P, N_DB, D], BF16, tag="w_all", bufs=2,
                                name=f"w_all_{nm}")
                nc.sync.dma_start(
                    out=w_all, in_=w_d[nm][:, :].rearrange("(a p) e -> p a e", p=P))
                return w_all

            def load_xt(nm, ic):
                xt_sb = pp.tile([P, N_DB, 512], BF16, tag="xt", bufs=3,
                                name=f"xt_{nm}_{ic}")
                nc.sync.dma_start(
                    out=xt_sb,
                    in_=x_tr[nm][:, ts(ic, 512)].rearrange("(a p) t -> p a t", p=P))
                return xt_sb

            def proj_qk(nm, dstT, w_all, xt_sb, ic):
                """dstT[:, eb, chunk ic] in fp8, LN+bias folded."""
                for eb in range(N_EB):
                    ps = mm_ps.tile([P, 512], F32, tag="mm",
                                    name=f"ps_{nm}_{ic}_{eb}")
                    for db in range(N_DB):
                        nc.tensor.matmul(ps, lhsT=w_all[:, db, ts(eb, P)],
                                         rhs=xt_sb[:, db, :],
                                         start=(db == 0), stop=False)
                    nc.tensor.matmul(ps, lhsT=corr_w[nm][:, ts(eb, P)],
                                     rhs=corr_rows[nm][:, ts(ic, 512)],
                                     start=False, stop=True)
                    nc.vector.tensor_mul(out=dstT[:, eb, ts(ic, 512)], in0=ps,
                                         in1=rstd_bc[nm][:, ts(ic, 512)])

            def proj_v(w_all, xt_sb, ic):
                for ec in range(N_EC):
                    for ml in range(4):
                        m = 4 * ic + ml
                        ps = mm_ps.tile([P, 512], F32, tag="mm",
                                        name=f"ps_v_{m}_{ec}")
                        for db in range(N_DB):
                            nc.tensor.matmul(ps,
                                             lhsT=xt_sb[:, db, ds(ml * P, P)],
                                             rhs=w_all[:, db, ts(ec, 512)],
                                             start=(db == 0), stop=False)
                        nc.tensor.matmul(ps, lhsT=corr_rows["v"][:, ts(m, P)],
                                         rhs=corr_w["v"][:, ts(ec, 512)],
                                         start=False, stop=True)
                        nc.scalar.activation(out=v_sb[:, m, ts(ec, 512)], in_=ps,
                                             func=AF.Identity,
                                             scale=rstd_cols_v[:, m:m + 1])

            # Each tensor's stats chain is emitted one projection-phase early
            # so the in-order DVE/PE queues have it ready when the projection
            # needs corr rows (avoids a stall at each phase transition).
            w_k = load_w("k")
            with nc.named_scope("proj_k"):
                for ic in range(N_IC):
                    xt_sb = load_xt("k", ic)
                    stats_chunk("k", ic)
                    proj_qk("k", kT, w_k, xt_sb, ic)
                    stats_chunk("t", ic)
            w_q = load_w("t")
            with nc.named_scope("proj_q"):
                for ic in range(N_IC):
                    xt_sb = load_xt("t", ic)
                    proj_qk("t", qT, w_q, xt_sb, ic)
                    stats_chunk("v", ic)
            w_v = load_w("v")
            with nc.named_scope("proj_v"):
                for ic in range(N_IC):
                    xt_sb = load_xt("v", ic)
                    proj_v(w_v, xt_sb, ic)

        # ---- attention ----
        attv_ps = ctx.enter_context(tc.tile_pool(name="attv_ps", bufs=2, space="PSUM"))
        with tc.tile_pool(name="att", bufs=1) as att:
            for ic in range(N_IC):
                with nc.named_scope(f"scores_{ic}"):
                    aT = att.tile([P, N_IT, 512], BF16, tag="aT", bufs=2,
                                  name=f"aT_{ic}")
                    for jt in range(N_IT):
                        ps = mm_ps.tile([P, 512], F32, tag="mm",
                                        name=f"ps_s_{ic}_{jt}")
                        for ebp in range(N_EB // 2):
                            nc.tensor.matmul(
                                ps, lhsT=kT[:, ds(2 * ebp, 2), ts(jt, P)],
                                rhs=qT[:, ds(2 * ebp, 2), ts(ic, 512)],
                                start=(ebp == 0), stop=(ebp == N_EB // 2 - 1),
                                perf_mode=mybir.MatmulPerfMode.DoubleRow)
                        nc.scalar.activation(
                            out=aT[:, jt, :], in_=ps,
                            func=AF.Exp, scale=SCALE)
                with nc.named_scope(f"attv_{ic}"):
                    for isub in range(4):
                        ou = attv_ps.tile([P, D], F32, tag="ou",
                                          name=f"ou_{ic}_{isub}")
                        zz = mm_ps.tile([P, 1], F32, tag="mm",
                                        name=f"z_{ic}_{isub}")
                        # same-bank runs of 16 accumulating matmuls (bank cycling
                        # between consecutive matmuls forces PE micro-stalls)
                        for ec in range(N_EC):
                            for jt in range(N_IT):
                                nc.tensor.matmul(
                                    ou[:, ts(ec, 512)],
                                    lhsT=aT[:, jt, ts(isub, P)],
                                    rhs=v_sb[:, jt, ts(ec, 512)],
                                    start=(jt == 0), stop=(jt == N_IT - 1))
                        for jt in range(N_IT):
                            nc.tensor.matmul(zz, lhsT=aT[:, jt, ts(isub, P)],
                                             rhs=ones_t,
                                             start=(jt == 0), stop=(jt == N_IT - 1))
                        rz = att.tile([P, 1], F32, tag="rz", bufs=2,
                                      name=f"rz_{ic}_{isub}")
                        nc.vector.reciprocal(out=rz, in_=zz)
                        o_sb = att.tile([P, D], F32, tag="o_sb", bufs=2,
                                        name=f"o_{ic}_{isub}")
                        nc.vector.tensor_scalar_mul(out=o_sb, in0=ou, scalar1=rz)
                        nc.sync.dma_start(out=out[ts(ic * 4 + isub, P), :], in_=o_sb)

    nc.compile()
    return nc


_NC_CACHE = None


def _get_module():
    global _NC_CACHE
    if _NC_CACHE is None:
        _NC_CACHE = build_module()
    return _NC_CACHE


def host_prep(target, source_k, source_v, Wq, bq, Wk, bk, Wv, bv,
              g_t, b_t, g_k, b_k, g_v, b_v):
    """Shared host-side input prep; returns per-core in_maps."""
    bf16 = ml_dtypes.bfloat16
    f32 = np.float32
    Wq = np.asarray(Wq, f32); bq = np.asarray(bq, f32)
    Wk = np.asarray(Wk, f32); bk = np.asarray(bk, f32)
    Wv = np.asarray(Wv, f32); bv = np.asarray(bv, f32)
    g_t = np.asarray(g_t, f32); b_t = np.asarray(b_t, f32)
    g_k = np.asarray(g_k, f32); b_k = np.asarray(b_k, f32)
    g_v = np.asarray(g_v, f32); b_v = np.asarray(b_v, f32)

    # Fold the layernorm affine (g, b) into the projection weights/biases:
    #   LN_affine(x) @ W.T + b  ==  LN_plain(x) @ (W*g).T + (b + W @ b_ln)
    wts = {"t": np.ascontiguousarray((Wq * g_t[None, :]).T).astype(bf16),
           "k": np.ascontiguousarray((Wk * g_k[None, :]).T).astype(bf16),
           "v": np.ascontiguousarray((Wv * g_v[None, :]).T).astype(bf16)}
    bias = {"t": (bq + Wq @ b_t).astype(bf16), "k": (bk + Wk @ b_k).astype(bf16),
            "v": (bv + Wv @ b_v).astype(bf16)}
    csum = {nm: wts[nm].astype(f32).sum(axis=0).astype(bf16) for nm in wts}
    ident = np.eye(P, dtype=f32)

    xs = {"t": np.asarray(target, f32), "k": np.asarray(source_k, f32),
          "v": np.asarray(source_v, f32)}
    in_maps = []
    for b in range(B):
        im = {"ident": ident}
        for nm in ("t", "k", "v"):
            im[f"x_{nm}"] = np.ascontiguousarray(xs[nm][b]).astype(bf16)
            im[f"xt_{nm}"] = np.ascontiguousarray(xs[nm][b].T).astype(bf16)
            im[f"w_{nm}"] = wts[nm]
            im[f"cs_{nm}"] = csum[nm]
            im[f"b_{nm}"] = bias[nm]
        in_maps.append(im)
    return in_maps


def kernel(target, source_k, source_v, Wq, bq, Wk, bk, Wv, bv,
           g_t, b_t, g_k, b_k, g_v, b_v):
    in_maps = host_prep(target, source_k, source_v, Wq, bq, Wk, bk, Wv, bv,
                        g_t, b_t, g_k, b_k, g_v, b_v)
    nc = _get_module()
    res = run_bass_kernel_spmd(nc, in_maps, core_ids=list(range(B)),
                               trace=bool(int(os.environ.get("KERNEL_TRACE", "0"))))
    out = np.stack([res.results[b]["out"] for b in range(B)], axis=0)
    kernel.last_results = res
    return out
